# revision 4
# baseline (speedup 1.0000x reference)
"""Hybrid gather kernel: PE one-hot streaming for h[src], SWDGE for h[dst].

Edges are globally sorted by src on the host and sharded contiguously, so
each core's src rows are monotone: each 128-edge tile's src values span at
most TSPAN consecutive 128-row blocks of h. The host stages, per tile, the
TSPAN candidate h-blocks (stationary operands) and block-selective one-hot
matrices; the device reconstructs h[src]^T feature-major with TSPAN
accumulating matmuls per tile — no Pool engine involvement. h[dst] (arbitrary
indices) uses the indirect-DMA path: 216 calls/core of serialized Pool
descriptor-gen == the critical path. Everything else (fp16 MLP, transposes,
activations) overlaps underneath. Output is computed in src-sorted order and
un-permuted on the host.

Measured (2026-08-08): rel err 5.80e-04 on the full 8-core HW run; exec
332695 ns (cost-model timeline with the indirect-DMA per-call cost
recalibrated to the HW wall-clock slope) vs 653144 ns baseline -> 1.96x.

HW facts behind the design (probed on this image, see forspeed*.py /
probe_multi.py):
  - One indirect-DMA call moves at most 128 rows (one descriptor per dest
    SBUF partition). Multi-index forms ([128,K] idx, 2D/3D contiguous or
    strided dest) silently corrupt data on HW even though CoreSim models
    them correctly: the unroller advances the source linearly with the dest
    pitch instead of walking the offset list.
  - Per-call cost is ~1.50-1.66us of serialized Pool-engine time, flat in
    bytes/call (64KB vs 16KB identical) and in queue count (num_swdge_queues
    2/4 correct but zero speedup) -> desc-gen launch overhead, engine-serial.
    The stock cost model's 994ns fixed overhead underestimates it by ~1.5x.
  - InstDMAGatherAnt (thousands of rows/call) needs the HIPI GPSIMD ucode
    library, absent from this bedrock image (NRT_EXEC_UNIT_UNRECOVERABLE).
So any design needs >= E/128 = 216 calls/core for the one arbitrary-side
permutation (~333us); the sorted side streams for free under it. 432 -> 216
calls is the whole win; the remaining floor is the per-call launch cost.
"""

import numpy as np

import concourse.mybir as mybir
import concourse.tile as tile
from concourse import bacc
from concourse.bass import IndirectOffsetOnAxis
from concourse.bass_utils import run_bass_kernel_spmd
from concourse.masks import make_identity

N_NODES = 200000
D = 128
E_TOTAL = 220000
N_CORES = 8
E_CORE = E_TOTAL // N_CORES            # 27500
TILE_E = 128
TILES = 216                            # ceil(27500/128) -> 27648
E_PAD = TILES * TILE_E                 # 27648
TSPAN = 4                              # candidate h-blocks per tile
CHUNK_TILES = 4
CHUNK_E = CHUNK_TILES * TILE_E         # 512
NCHUNKS = TILES // CHUNK_TILES         # 54
NBLK_H = (N_NODES + 127) // 128 + TSPAN - 1   # padded h16 blocks (1566)

F32 = mybir.dt.float32
F16 = mybir.dt.float16
I32 = mybir.dt.int32


def build_nc2(tiles=TILES):
    nchunks = tiles // CHUNK_TILES
    e_pad = tiles * TILE_E
    nc = bacc.Bacc("TRN2", target_bir_lowering=False, debug=False)

    hfull = nc.dram_tensor("hfull", [NBLK_H * 128, D], F16, kind="ExternalInput")
    hst = nc.dram_tensor("hst", [128, tiles * TSPAN * D], F16, kind="ExternalInput")
    oh = nc.dram_tensor("oh", [128, tiles * TSPAN * TILE_E], F16, kind="ExternalInput")
    didx = nc.dram_tensor("didx", [128, tiles], I32, kind="ExternalInput")
    w1a = nc.dram_tensor("w1a", [128, 128], F16, kind="ExternalInput")
    w1b = nc.dram_tensor("w1b", [128, 128], F16, kind="ExternalInput")
    w2 = nc.dram_tensor("w2", [128, 128], F16, kind="ExternalInput")
    w3 = nc.dram_tensor("w3", [128, 2], F16, kind="ExternalInput")
    b1 = nc.dram_tensor("b1", [128, 1], F32, kind="ExternalInput")
    b2 = nc.dram_tensor("b2", [128, 1], F32, kind="ExternalInput")
    b3x2 = nc.dram_tensor("b3x2", [2, 1], F32, kind="ExternalInput")
    out = nc.dram_tensor("out", [2, e_pad], F32, kind="ExternalOutput")

    Relu = mybir.ActivationFunctionType.Relu

    with tile.TileContext(nc) as tc:
        with (
            tc.tile_pool(name="const", bufs=1) as cp,
            tc.tile_pool(name="strm", bufs=4) as sp,
            tc.tile_pool(name="gat", bufs=6) as gp,
            tc.tile_pool(name="act", bufs=4) as ap,
            tc.tile_pool(name="outp", bufs=4) as op,
            tc.tile_pool(name="selp", bufs=2, space="PSUM") as sel_pool,
            tc.tile_pool(name="trp", bufs=2, space="PSUM") as tr_pool,
            tc.tile_pool(name="l1p", bufs=1, space="PSUM") as l1_pool,
            tc.tile_pool(name="l2p", bufs=1, space="PSUM") as l2_pool,
        ):
            ident = cp.tile([128, 128], F16)
            make_identity(nc, ident[:])
            w1a_t = cp.tile([128, 128], F16)
            nc.sync.dma_start(w1a_t[:], w1a.ap())
            w1b_t = cp.tile([128, 128], F16)
            nc.sync.dma_start(w1b_t[:], w1b.ap())
            w2_t = cp.tile([128, 128], F16)
            nc.sync.dma_start(w2_t[:], w2.ap())
            w3_t = cp.tile([128, 2], F16)
            nc.sync.dma_start(w3_t[:], w3.ap())
            b1_t = cp.tile([128, 1], F32)
            nc.sync.dma_start(b1_t[:], b1.ap())
            b2_t = cp.tile([128, 1], F32)
            nc.sync.dma_start(b2_t[:], b2.ap())
            b3_t = cp.tile([2, 1], F32)
            nc.sync.dma_start(b3_t[:], b3x2.ap())
            didx_t = cp.tile([128, tiles], I32)
            nc.sync.dma_start(didx_t[:], didx.ap())

            out_ap = out.ap()
            CW = CHUNK_TILES * TSPAN * 128   # hst/oh cols per chunk (2048)

            for c in range(nchunks):
                # staged stationaries + one-hots for this chunk's 4 tiles
                hst_c = sp.tile([128, CW], F16, tag="hst")
                nc.sync.dma_start(hst_c[:], hst.ap()[:, c * CW:(c + 1) * CW])
                oh_c = sp.tile([128, CW], F16, tag="oh")
                nc.sync.dma_start(oh_c[:], oh.ap()[:, c * CW:(c + 1) * CW])

                # dst gather: 4 indirect calls (the Pool critical path)
                xd = gp.tile([128, CHUNK_TILES, D], F16, tag="xd")
                for i in range(CHUNK_TILES):
                    t = c * CHUNK_TILES + i
                    nc.gpsimd.indirect_dma_start(
                        out=xd[:, i, :],
                        out_offset=None,
                        in_=hfull.ap(),
                        in_offset=IndirectOffsetOnAxis(
                            ap=didx_t[:, t:t + 1], axis=0),
                    )

                # src rows via one-hot select matmuls -> feature-major psum
                sel = sel_pool.tile([128, CHUNK_E], F32, tag="sel", space="PSUM")
                for i in range(CHUNK_TILES):
                    for s in range(TSPAN):
                        col = (i * TSPAN + s) * 128
                        nc.tensor.matmul(
                            sel[:, i * TILE_E:(i + 1) * TILE_E],
                            hst_c[:, col:col + 128],
                            oh_c[:, col:col + 128],
                            start=(s == 0), stop=(s == TSPAN - 1))

                # dst tiles: transpose to feature-major
                trp = tr_pool.tile([128, CHUNK_E], F16, tag="trp", space="PSUM")
                for i in range(CHUNK_TILES):
                    nc.tensor.transpose(
                        trp[:, i * TILE_E:(i + 1) * TILE_E], xd[:, i, :], ident[:])

                xsT = ap.tile([128, CHUNK_E], F16, tag="xsT")
                nc.vector.tensor_copy(xsT[:], sel[:])
                xdT = ap.tile([128, CHUNK_E], F16, tag="xdT")
                nc.vector.tensor_copy(xdT[:], trp[:])

                # layer 1 (fwd | rev)
                l1 = l1_pool.tile([128, 2 * CHUNK_E], F32, tag="l1", space="PSUM")
                nc.tensor.matmul(l1[:, 0:CHUNK_E], w1a_t[:], xsT[:], start=True, stop=False)
                nc.tensor.matmul(l1[:, 0:CHUNK_E], w1b_t[:], xdT[:], start=False, stop=True)
                nc.tensor.matmul(l1[:, CHUNK_E:], w1a_t[:], xdT[:], start=True, stop=False)
                nc.tensor.matmul(l1[:, CHUNK_E:], w1b_t[:], xsT[:], start=False, stop=True)
                h1 = ap.tile([128, 2 * CHUNK_E], F16, tag="h1")
                nc.scalar.activation(h1[:], l1[:], Relu, bias=b1_t[:, 0:1])

                # layer 2
                l2 = l2_pool.tile([128, 2 * CHUNK_E], F32, tag="l2", space="PSUM")
                nc.tensor.matmul(l2[:, 0:CHUNK_E], w2_t[:], h1[:, 0:CHUNK_E],
                                 start=True, stop=True)
                nc.tensor.matmul(l2[:, CHUNK_E:], w2_t[:], h1[:, CHUNK_E:],
                                 start=True, stop=True)
                h2 = ap.tile([128, 2 * CHUNK_E], F16, tag="h2")
                nc.scalar.activation(h2[:], l2[:], Relu, bias=b2_t[:, 0:1])

                # layer 3: fwd+rev accumulate into [2, 512]
                l3 = l2_pool.tile([2, CHUNK_E], F32, tag="l2", space="PSUM")
                nc.tensor.matmul(l3[:], w3_t[:], h2[:, 0:CHUNK_E], start=True, stop=False)
                nc.tensor.matmul(l3[:], w3_t[:], h2[:, CHUNK_E:], start=False, stop=True)

                o = op.tile([2, CHUNK_E], F32, tag="o")
                nc.vector.tensor_scalar_add(o[:], l3[:], b3_t[:, 0:1])
                nc.sync.dma_start(out_ap[:, c * CHUNK_E:(c + 1) * CHUNK_E], o[:])

    nc.compile()
    return nc


def make_in_map2(h16p, src_s, dst_s, W1, b1, W2, b2, W3, b3, tiles=TILES):
    """h16p: fp16 h padded to NBLK_H*128 rows. src_s/dst_s: this core's
    edges sorted by src, length <= tiles*128 (padded here)."""
    e_pad = tiles * TILE_E
    n = len(src_s)
    src = np.empty(e_pad, np.int64)
    dst = np.empty(e_pad, np.int64)
    src[:n] = src_s
    dst[:n] = dst_s
    src[n:] = src_s[-1]
    dst[n:] = 0

    jt = np.arange(e_pad)
    t_of = jt // TILE_E
    lo_t = (src[::TILE_E] // 128).astype(np.int64)          # [tiles]
    s_of = src // 128 - lo_t[t_of]
    if s_of.min() < 0 or s_of.max() >= TSPAN:
        raise ValueError(f"tile block span {s_of.max()+1} exceeds TSPAN={TSPAN}")

    # stationaries: hst[p, ((t*TSPAN+s)*D)+f] = h16p[(lo_t+s)*128+p, f]
    blocks = (lo_t[:, None] + np.arange(TSPAN)[None, :])    # [tiles, TSPAN]
    hb = h16p.reshape(-1, 128, D)[blocks]                   # [tiles,TSPAN,128,D]
    hst = np.ascontiguousarray(
        hb.transpose(2, 0, 1, 3).reshape(128, tiles * TSPAN * D))

    # one-hots: oh[p, (t*TSPAN+s)*128 + jm] = (src_j in block lo_t+s, src_j%128==p)
    oh = np.zeros((128, tiles * TSPAN * TILE_E), np.float16)
    p_of = (src % 128).astype(np.int64)
    colidx = (t_of * TSPAN + s_of) * TILE_E + (jt % TILE_E)
    oh[p_of, colidx] = 1.0

    didx = np.ascontiguousarray(dst.reshape(tiles, TILE_E).T, np.int32)

    return {
        "hfull": h16p,
        "hst": hst,
        "oh": oh,
        "didx": didx,
        "w1a": np.ascontiguousarray(W1[:128], np.float16),
        "w1b": np.ascontiguousarray(W1[128:], np.float16),
        "w2": np.ascontiguousarray(W2, np.float16),
        "w3": np.ascontiguousarray(W3, np.float16),
        "b1": np.ascontiguousarray(b1.reshape(128, 1), np.float32),
        "b2": np.ascontiguousarray(b2.reshape(128, 1), np.float32),
        "b3x2": np.ascontiguousarray((2.0 * b3).reshape(2, 1), np.float32),
    }


_NC_CACHE = {}


def _get_nc(tiles=TILES):
    if tiles not in _NC_CACHE:
        _NC_CACHE[tiles] = build_nc2(tiles)
    return _NC_CACHE[tiles]


def kernel(h, src, dst, W1, b1, W2, b2, W3, b3, **run_kwargs):
    h = np.asarray(h, np.float32)
    src = np.asarray(src).astype(np.int64)
    dst = np.asarray(dst).astype(np.int64)
    W1 = np.asarray(W1); W2 = np.asarray(W2); W3 = np.asarray(W3)
    b1 = np.asarray(b1); b2 = np.asarray(b2); b3 = np.asarray(b3)

    h16p = np.zeros((NBLK_H * 128, D), np.float16)
    h16p[:N_NODES] = h.astype(np.float16)

    order = np.argsort(src, kind="stable")
    src_s_all = src[order]
    dst_s_all = dst[order]

    nc = _get_nc()
    in_maps = []
    for c in range(N_CORES):
        sl = slice(c * E_CORE, (c + 1) * E_CORE)
        in_maps.append(make_in_map2(
            h16p, src_s_all[sl], dst_s_all[sl], W1, b1, W2, b2, W3, b3))

    try:
        res = run_bass_kernel_spmd(nc, in_maps, core_ids=list(range(N_CORES)),
                                   **run_kwargs)
    except Exception:
        import time as _time
        _time.sleep(5)
        res = run_bass_kernel_spmd(nc, in_maps, core_ids=list(range(N_CORES)),
                                   **run_kwargs)

    out_sorted = np.empty((E_TOTAL, 2), np.float32)
    for c in range(N_CORES):
        o = res.results[c]["out"]              # [2, E_PAD]
        out_sorted[c * E_CORE:(c + 1) * E_CORE] = o.T[:E_CORE]
    out = np.empty((E_TOTAL, 2), np.float32)
    out[order] = out_sorted
    if run_kwargs:
        kernel.last_results = res
    return out
